# revision 1
# baseline (speedup 1.0000x reference)
"""GAT layer (8-head attention + 4-hop propagation + LayerNorm) on 8 TRN2 cores.

Sharding: data-parallel over batch B=8 — core b computes batch b entirely
(no collectives). Per-core program (all f32 I/O, E/z dtype configurable):

  qT/kT = Wq/Wk @ x^T + b      [512, 1024]  (hidden on partitions)
  v01   = 0.1*(x @ Wv^T + bv)  [1024, 512]  (nodes on partitions)
  per head h (64-dim slices of qT/kT/v01):
    E[m, n] = exp(k_h[m]·q_h[n]) * mask[m, n]          (scores transposed)
    D[n]    = sum_m E[m, n]      (ones-column matmul, fused with hop 1)
    z1 = (E.T @ v01_h) * (9/D)  + v01_h                (= 0.9*P@v + 0.1*v)
    z_{t+1} = (E.T @ z_t) * (0.9/D) + v01_h            (hops 2..4)
    y[:, h*64:+64] = z4
  out = LayerNorm(y + x) * gamma + beta

The softmax max-subtraction is skipped: scores are bounded (~22) so exp()
cannot overflow fp32, and exp(MASK)=0 exactly matches the reference's
masked softmax (verified rel err 5.5e-6 vs jax reference in f32).
"""

import numpy as np

import concourse.bass as bass
import concourse.mybir as mybir
import concourse.tile as tile
from concourse import bacc
from concourse.bass_utils import run_bass_kernel_spmd

B, N, H = 8, 1024, 512
NHEADS, U = 8, 64
P = 128
NT = N // P          # 8 node tiles
KT = H // P          # 4 hidden tiles
ALPHA = 0.1
LN_EPS = 1e-5
F32 = mybir.dt.float32

# E/z dtype for the propagation path: "float32" (rel err ~6e-6) or
# "bfloat16" (rel err ~1.2e-3, ~2x faster weight loads on the hop matmuls).
E_DTYPE_NAME = "bfloat16"

_BUILD_CACHE = {}


def build_nc(edt_name: str, apply_affine: bool):
    key = (edt_name, apply_affine)
    if key in _BUILD_CACHE:
        return _BUILD_CACHE[key]

    EDT = getattr(mybir.dt, edt_name)
    ZDT = EDT
    nc = bacc.Bacc(None, target_bir_lowering=False)

    xT_d = nc.dram_tensor("xT", [H, N], F32, kind="ExternalInput")
    xn_d = nc.dram_tensor("xn", [N, H], F32, kind="ExternalInput")
    maskT_d = nc.dram_tensor("maskT", [N, N], EDT, kind="ExternalInput")
    wq_d = nc.dram_tensor("wqT", [H, H], F32, kind="ExternalInput")
    wk_d = nc.dram_tensor("wkT", [H, H], F32, kind="ExternalInput")
    wv_d = nc.dram_tensor("wvT01", [H, H], F32, kind="ExternalInput")
    bq_d = nc.dram_tensor("bq", [H], F32, kind="ExternalInput")
    bk_d = nc.dram_tensor("bk", [H], F32, kind="ExternalInput")
    bv_d = nc.dram_tensor("bv01", [H], F32, kind="ExternalInput")
    if apply_affine:
        gam_d = nc.dram_tensor("gammar", [P, H], F32, kind="ExternalInput")
        bet_d = nc.dram_tensor("betar", [P, H], F32, kind="ExternalInput")
    out_d = nc.dram_tensor("out", [N, H], F32, kind="ExternalOutput")

    with tile.TileContext(nc) as tc:
        with tc.tile_pool(name="const", bufs=1) as cpool, \
             tc.tile_pool(name="big", bufs=1) as bpool, \
             tc.tile_pool(name="epool", bufs=2 if edt_name == "bfloat16" else 1) as epool, \
             tc.tile_pool(name="zpool", bufs=3) as zpool, \
             tc.tile_pool(name="tpool", bufs=2) as tpool, \
             tc.tile_pool(name="spool", bufs=4) as spool, \
             tc.tile_pool(name="ps512", bufs=2, space="PSUM") as ps512, \
             tc.tile_pool(name="scps", bufs=2, space="PSUM") as scps, \
             tc.tile_pool(name="dps", bufs=2, space="PSUM") as dpsp:

            # ---- persistent SBUF residents ----
            # Loads spread over the sync/gpsimd/scalar/vector DMA queues so
            # the PE-critical tensors (xT + weights) land first in parallel.
            bq_sb = cpool.tile([P, KT], F32)
            nc.gpsimd.dma_start(bq_sb[:], bq_d[:].rearrange("(t p) -> p t", p=P))
            bk_sb = cpool.tile([P, KT], F32)
            nc.scalar.dma_start(bk_sb[:], bk_d[:].rearrange("(t p) -> p t", p=P))
            # bv01 as a [1, 512] row for the rank-1 bias matmul
            bvrow_sb = cpool.tile([P, H], F32, tag="bvrow")
            nc.scalar.dma_start(bvrow_sb[:1, :], bv_d[:].rearrange("(a h) -> a h", a=1))
            maskT_sb = cpool.tile([P, NT, N], EDT)
            xn_sb = cpool.tile([P, NT, H], F32)
            ones_row = cpool.tile([P, P], F32, tag="onesrow")
            nc.vector.memset(ones_row[:1, :], 1.0)
            eps_sb = cpool.tile([P, 1], F32, tag="eps")
            nc.vector.memset(eps_sb[:], LN_EPS)
            if apply_affine:
                gam_sb = cpool.tile([P, H], F32, tag="gam")
                bet_sb = cpool.tile([P, H], F32, tag="bet")

            qT_sb = bpool.tile([P, KT, N], F32, tag="qT")
            kT_sb = bpool.tile([P, KT, N], F32, tag="kT")
            # v01 per-head blocks of 64 values + a trailing 1.0 column; the
            # ones column rides hop 1's moving operand to produce D in PSUM.
            v01_sb = bpool.tile([P, NT, NHEADS, U + 1], ZDT, tag="v01")
            nc.vector.memset(v01_sb[:, :, :, U:U + 1], 1.0)
            y_sb = bpool.tile([P, NT, H], F32, tag="y")

            # ---- phase 1: projections (inside a scope so xT/W free early) ----
            with tc.tile_pool(name="ph1", bufs=1) as p1:
                xT_sb = p1.tile([P, KT, N], F32, tag="xT")
                nc.sync.dma_start(xT_sb[:], xT_d[:, :].rearrange("(t p) n -> p t n", p=P))
                wq_sb = p1.tile([P, KT, H], F32, tag="wq")
                nc.gpsimd.dma_start(wq_sb[:], wq_d[:, :].rearrange("(t p) i -> p t i", p=P))
                wk_sb = p1.tile([P, KT, H], F32, tag="wk")
                nc.scalar.dma_start(wk_sb[:], wk_d[:, :].rearrange("(t p) i -> p t i", p=P))
                wv_sb = p1.tile([P, KT, H], F32, tag="wv")
                nc.scalar.dma_start(wv_sb[:], wv_d[:, :].rearrange("(t p) i -> p t i", p=P))
                # big non-critical loads queue up behind the critical ones
                nc.sync.dma_start(maskT_sb[:], maskT_d[:, :].rearrange("(t p) n -> p t n", p=P))
                nc.gpsimd.dma_start(xn_sb[:], xn_d[:, :].rearrange("(t p) h -> p t h", p=P))
                if apply_affine:
                    nc.gpsimd.dma_start(gam_sb[:], gam_d[:, :])
                    nc.gpsimd.dma_start(bet_sb[:], bet_d[:, :])

                # qT[i, n] = sum_k WqT[k, i] xT[k, n] + bq[i]
                for w_sb, b_sb, dst in ((wq_sb, bq_sb, qT_sb), (wk_sb, bk_sb, kT_sb)):
                    for it in range(KT):
                        for ncx in range(2):
                            ps = ps512.tile([P, 512], F32, tag="ps512")
                            for kt in range(KT):
                                nc.tensor.matmul(
                                    ps[:],
                                    w_sb[:, kt, it * P:(it + 1) * P],
                                    xT_sb[:, kt, ncx * 512:(ncx + 1) * 512],
                                    start=(kt == 0), stop=(kt == KT - 1),
                                )
                            nc.vector.tensor_scalar_add(
                                dst[:, it, ncx * 512:(ncx + 1) * 512], ps[:],
                                b_sb[:, it:it + 1],
                            )

                # v01[node, j] = sum_k xT[k, node] WvT01[k, j] + bv01[j]
                for nt in range(NT):
                    ps = ps512.tile([P, 512], F32, tag="ps512")
                    nc.tensor.matmul(
                        ps[:], ones_row[:1, :P], bvrow_sb[:1, :],
                        start=True, stop=False,
                    )
                    for kt in range(KT):
                        nc.tensor.matmul(
                            ps[:],
                            xT_sb[:, kt, nt * P:(nt + 1) * P],
                            wv_sb[:, kt, :],
                            start=False, stop=(kt == KT - 1),
                        )
                    nc.scalar.activation(
                        v01_sb[:, nt, :, 0:U],
                        ps[:].rearrange("p (h u) -> p h u", u=U),
                        mybir.ActivationFunctionType.Copy,
                    )

            # ---- phase 2: per-head attention + propagation ----
            for h in range(NHEADS):
                pt, po = h // 2, (h % 2) * U
                kh = kT_sb[po:po + U, pt, :]   # [64, 1024] (d on partitions)
                qh = qT_sb[po:po + U, pt, :]

                e_sb = epool.tile([P, NT, N], EDT, tag="E")
                for mt in range(NT):
                    sps = scps.tile([P, N], F32, tag="scps")
                    for ncx in range(2):
                        nc.tensor.matmul(
                            sps[:, ncx * 512:(ncx + 1) * 512],
                            kh[:, mt * P:(mt + 1) * P],
                            qh[:, ncx * 512:(ncx + 1) * 512],
                            start=True, stop=True,
                        )
                    nc.scalar.activation(
                        e_sb[:, mt, :], sps[:], mybir.ActivationFunctionType.Exp,
                    )
                    nc.vector.tensor_tensor(
                        e_sb[:, mt, :], e_sb[:, mt, :], maskT_sb[:, mt, :],
                        mybir.AluOpType.mult,
                    )

                w0 = v01_sb[:, :, h, 0:U]  # [128, 8, 64]
                rd09 = spool.tile([P, NT], F32, tag="rd09")
                rd9 = spool.tile([P, NT], F32, tag="rd9")
                z_prev = None
                for hop in range(4):
                    t = tpool.tile([P, NT, U], F32, tag="t")
                    if hop == 0:
                        # moving operand carries [z0 | 1]; D lands in col U.
                        # Two 1-bank psum tiles: a 65-col accumulation group
                        # cannot cross a PSUM bank boundary.
                        halves = [
                            dpsp.tile([P, NT // 2, U + 1], F32, tag="hps1",
                                      name=f"hps1_{h}_{i}")
                            for i in (0, 1)
                        ]
                        for nt in range(NT):
                            hp = halves[nt // 4]
                            for mt in range(NT):
                                nc.tensor.matmul(
                                    hp[:, nt % 4, :],
                                    e_sb[:, mt, nt * P:(nt + 1) * P],
                                    v01_sb[:, mt, h, :],
                                    start=(mt == 0), stop=(mt == NT - 1),
                                )
                        rdraw = spool.tile([P, NT], F32, tag="rdraw")
                        nc.vector.reciprocal(rdraw[:, 0:4], halves[0][:, :, U])
                        nc.vector.reciprocal(rdraw[:, 4:8], halves[1][:, :, U])
                        nc.vector.tensor_scalar_mul(rd09[:], rdraw[:], 1.0 - ALPHA)
                        nc.vector.tensor_scalar_mul(rd9[:], rdraw[:],
                                                    (1.0 - ALPHA) / ALPHA)
                        for i in (0, 1):
                            nc.vector.tensor_tensor(
                                t[:, 4 * i:4 * (i + 1), :],
                                halves[i][:, :, 0:U],
                                rd9[:, 4 * i:4 * (i + 1), None].to_broadcast(
                                    [P, 4, U]),
                                mybir.AluOpType.mult,
                            )
                    else:
                        hps = ps512.tile([P, NT, U], F32, tag="ps512")
                        for nt in range(NT):
                            for mt in range(NT):
                                nc.tensor.matmul(
                                    hps[:, nt, :],
                                    e_sb[:, mt, nt * P:(nt + 1) * P],
                                    z_prev[:, mt, :],
                                    start=(mt == 0), stop=(mt == NT - 1),
                                )
                        scale = rd09[:, :, None].to_broadcast([P, NT, U])
                        nc.vector.tensor_tensor(t[:], hps[:], scale,
                                                mybir.AluOpType.mult)
                    if hop == 3:
                        out_ap = y_sb[:, :, h * U:(h + 1) * U]
                    else:
                        znew = zpool.tile([P, NT, U], ZDT, tag="z")
                        out_ap = znew[:]
                    nc.vector.tensor_tensor(out_ap, t[:], w0, mybir.AluOpType.add)
                    if hop != 3:
                        z_prev = znew

            # ---- phase 3: residual + LayerNorm ----
            for nt in range(NT):
                yt = y_sb[:, nt, :]
                nc.vector.tensor_tensor(yt, yt, xn_sb[:, nt, :], mybir.AluOpType.add)
                st6 = spool.tile([P, 6], F32, tag="st6")
                nc.vector.bn_stats(st6[:], yt)
                st2 = spool.tile([P, 2], F32, tag="st2")
                nc.vector.bn_aggr(st2[:], st6[:])
                sd = spool.tile([P, 1], F32, tag="sd")
                nc.scalar.activation(
                    sd[:], st2[:, 1:2], mybir.ActivationFunctionType.Sqrt,
                    bias=eps_sb[:, :],
                )
                rstd = spool.tile([P, 1], F32, tag="rstd")
                nc.vector.reciprocal(rstd[:], sd[:])
                nc.vector.tensor_scalar(
                    yt, yt, st2[:, 0:1], rstd[:],
                    mybir.AluOpType.subtract, mybir.AluOpType.mult,
                )
                if apply_affine:
                    nc.vector.tensor_tensor(yt, yt, gam_sb[:, :], mybir.AluOpType.mult)
                    nc.vector.tensor_tensor(yt, yt, bet_sb[:, :], mybir.AluOpType.add)
                nc.sync.dma_start(
                    out_d[:, :].rearrange("(t p) h -> p t h", p=P)[:, nt, :], yt)

    nc.finalize()
    _BUILD_CACHE[key] = nc
    return nc


def make_in_maps(x, adj, Wq, bq, Wk, bk, Wv, bv, gamma, beta, edt_name, apply_affine):
    import ml_dtypes
    np_edt = np.float32 if edt_name == "float32" else ml_dtypes.bfloat16
    x = np.ascontiguousarray(np.asarray(x, np.float32))
    adj = np.asarray(adj)
    wqT = np.ascontiguousarray(np.asarray(Wq, np.float32).T)
    wkT = np.ascontiguousarray(np.asarray(Wk, np.float32).T)
    wvT01 = np.ascontiguousarray((ALPHA * np.asarray(Wv, np.float32)).T)
    bq = np.asarray(bq, np.float32)
    bk = np.asarray(bk, np.float32)
    bv01 = ALPHA * np.asarray(bv, np.float32)
    in_maps = []
    for b in range(B):
        m = {
            "xT": np.ascontiguousarray(x[b].T),
            "xn": x[b],
            "maskT": np.ascontiguousarray((adj[b] != 0).T.astype(np_edt)),
            "wqT": wqT, "wkT": wkT, "wvT01": wvT01,
            "bq": bq, "bk": bk, "bv01": bv01,
        }
        if apply_affine:
            m["gammar"] = np.ascontiguousarray(
                np.broadcast_to(np.asarray(gamma, np.float32), (P, H)))
            m["betar"] = np.ascontiguousarray(
                np.broadcast_to(np.asarray(beta, np.float32), (P, H)))
        in_maps.append(m)
    return in_maps


def kernel(x, adj, Wq, bq, Wk, bk, Wv, bv, gamma, beta, _trace=False):
    apply_affine = not (
        np.allclose(np.asarray(gamma), 1.0) and np.allclose(np.asarray(beta), 0.0)
    )
    nc = build_nc(E_DTYPE_NAME, apply_affine)
    in_maps = make_in_maps(
        x, adj, Wq, bq, Wk, bk, Wv, bv, gamma, beta, E_DTYPE_NAME, apply_affine
    )
    res = run_bass_kernel_spmd(nc, in_maps, list(range(B)), trace=_trace)
    out = np.stack([np.asarray(res.results[b]["out"]) for b in range(B)])
    if _trace:
        return out.astype(np.float32), res
    return out.astype(np.float32)



# revision 3
# speedup vs baseline: 1.8707x; 1.8707x over previous
"""GAT layer (8-head attention + 4-hop propagation + LayerNorm) on 8 TRN2 cores.

Sharding: data-parallel over batch B=8 - core b computes batch b entirely
(no collectives). Per-core program (bf16 matmuls, f32 accumulation):

  qT/kT = bf16(Wq/Wk @ x^T + b)   [512, 1024]  (hidden on partitions)
  v01   = bf16(0.1 * x @ Wv^T)    [1024, 512]  (bias bv folded into xn)
  per head h (64-dim slices):
    E[m, n] = exp(k_h[m]·q_h[n]) * mask[m, n]   (scores transposed, bf16)
    D[n]    = sum_m E[m, n]    (ones-column rides hop-1 moving operand)
    z1 = (E.T @ v01_h) * (9/D) + v01_h
    z_{t+1} = (E.T @ z_t) * (0.9/D) + v01_h     (hops 2..4)
    y[:, h*64:+64] = z4
  out = LayerNorm(y + xn) * gamma + beta        (xn = x + bv, host-folded:
                                                 P rows sum to 1 so +bv
                                                 commutes through the hops)

Schedule: heads run in pairs (2p, 2p+1) sharing a qT/kT partition tile at
base partitions 0/64 so the K=64 score matmuls row-tile concurrently on
the PE. Scores for pair p+1 are sprinkled between the hop segments of
pair p so the scalar-engine exp (the serial elementwise bottleneck) stays
overlapped with PE matmul work. Mask-multiplies alternate vector/gpsimd.

fp32 matmuls on TRN2 run in fp32_mode=LOW_HIGH (two PE passes) - keeping
every matmul operand bf16 halves PE time vs the fp32 variant.
"""

import numpy as np

import concourse.bass as bass
import concourse.mybir as mybir
import concourse.tile as tile
from concourse import bacc
from concourse.bass_utils import run_bass_kernel_spmd

B, N, H = 8, 1024, 512
NHEADS, U = 8, 64
P = 128
NT = N // P          # 8 node tiles
KT = H // P          # 4 hidden tiles
NPAIR = NHEADS // 2
ALPHA = 0.1
LN_EPS = 1e-5
F32 = mybir.dt.float32
BF16 = mybir.dt.bfloat16

_BUILD_CACHE = {}


def build_nc(apply_affine: bool):
    if apply_affine in _BUILD_CACHE:
        return _BUILD_CACHE[apply_affine]

    nc = bacc.Bacc(None, target_bir_lowering=False)

    xT_d = nc.dram_tensor("xT", [H, N], BF16, kind="ExternalInput")
    xn_d = nc.dram_tensor("xn", [N, H], F32, kind="ExternalInput")
    maskT_d = nc.dram_tensor("maskT", [N, N], BF16, kind="ExternalInput")
    wq_d = nc.dram_tensor("wqT", [H, H], BF16, kind="ExternalInput")
    wk_d = nc.dram_tensor("wkT", [H, H], BF16, kind="ExternalInput")
    wv_d = nc.dram_tensor("wvT01", [H, H], BF16, kind="ExternalInput")
    bq_d = nc.dram_tensor("bq", [H], F32, kind="ExternalInput")
    bk_d = nc.dram_tensor("bk", [H], F32, kind="ExternalInput")
    if apply_affine:
        gam_d = nc.dram_tensor("gammar", [P, H], F32, kind="ExternalInput")
        bet_d = nc.dram_tensor("betar", [P, H], F32, kind="ExternalInput")
    out_d = nc.dram_tensor("out", [N, H], F32, kind="ExternalOutput")

    with tile.TileContext(nc) as tc:
        with tc.tile_pool(name="const", bufs=1) as cpool, \
             tc.tile_pool(name="big", bufs=1) as bpool, \
             tc.tile_pool(name="epool", bufs=4) as epool, \
             tc.tile_pool(name="zpool", bufs=3) as zpool, \
             tc.tile_pool(name="tpool", bufs=2) as tpool, \
             tc.tile_pool(name="spool", bufs=4) as spool, \
             tc.tile_pool(name="scps", bufs=4, space="PSUM") as scps, \
             tc.tile_pool(name="ps512", bufs=2, space="PSUM") as ps512, \
             tc.tile_pool(name="dps", bufs=2, space="PSUM") as dpsp:

            # ---- persistent SBUF residents ----
            bq_sb = cpool.tile([P, KT], F32)
            nc.gpsimd.dma_start(bq_sb[:], bq_d[:].rearrange("(t p) -> p t", p=P))
            bk_sb = cpool.tile([P, KT], F32)
            nc.scalar.dma_start(bk_sb[:], bk_d[:].rearrange("(t p) -> p t", p=P))
            maskT_sb = cpool.tile([P, NT, N], BF16)
            nc.sync.dma_start(maskT_sb[:], maskT_d[:, :].rearrange("(t p) n -> p t n", p=P))
            xn_sb = cpool.tile([P, NT, H], F32)
            nc.gpsimd.dma_start(xn_sb[:], xn_d[:, :].rearrange("(t p) h -> p t h", p=P))
            eps_sb = cpool.tile([P, 1], F32, tag="eps")
            nc.vector.memset(eps_sb[:], LN_EPS)
            if apply_affine:
                gam_sb = cpool.tile([P, H], F32, tag="gam")
                bet_sb = cpool.tile([P, H], F32, tag="bet")
                nc.gpsimd.dma_start(gam_sb[:], gam_d[:, :])
                nc.gpsimd.dma_start(bet_sb[:], bet_d[:, :])

            qT_sb = bpool.tile([P, KT, N], BF16, tag="qT")
            kT_sb = bpool.tile([P, KT, N], BF16, tag="kT")
            # v01 per-head 64-wide blocks + trailing 1.0 column: the ones
            # column rides hop 1's moving operand to produce D in PSUM.
            v01_sb = bpool.tile([P, NT, NHEADS, U + 1], BF16, tag="v01")
            nc.vector.memset(v01_sb[:, :, :, U:U + 1], 1.0)
            y_sb = bpool.tile([P, NT, H], F32, tag="y")

            e_tiles = {}
            rd09 = {}
            rd9 = {}
            z_prev = {}
            EXP = mybir.ActivationFunctionType.Exp

            def scores_slot(p, mt):
                a = 2 * p
                pt = p
                if mt == 0:
                    for h in (a, a + 1):
                        e_tiles[h] = epool.tile([P, NT, N], BF16, tag="E",
                                                name=f"E_{h}")
                pss = []
                for ncx in (0, 1):
                    for i, po in ((0, 0), (1, U)):
                        ps = scps.tile([P, 512], F32, tag="scps",
                                       name=f"sc_{a + i}_{mt}_{ncx}")
                        nc.tensor.matmul(
                            ps[:],
                            kT_sb[po:po + U, pt, mt * P:(mt + 1) * P],
                            qT_sb[po:po + U, pt, ncx * 512:(ncx + 1) * 512],
                            start=True, stop=True,
                        )
                        pss.append((a + i, ncx, ps))
                for h, ncx, ps in pss:
                    nc.scalar.activation(
                        e_tiles[h][:, mt, ncx * 512:(ncx + 1) * 512], ps[:], EXP,
                    )
                for i, h in enumerate((a, a + 1)):
                    eng = nc.vector if (mt + i) % 2 == 0 else nc.gpsimd
                    eng.tensor_tensor(
                        e_tiles[h][:, mt, :], e_tiles[h][:, mt, :],
                        maskT_sb[:, mt, :], mybir.AluOpType.mult,
                    )

            def hop0(h):
                e_sb = e_tiles[h]
                halves = [
                    dpsp.tile([P, NT // 2, U + 1], F32, tag="hps",
                              name=f"hps_{h}_{i}")
                    for i in (0, 1)
                ]
                for nt in range(NT):
                    hp = halves[nt // 4]
                    for mt in range(NT):
                        nc.tensor.matmul(
                            hp[:, nt % 4, :],
                            e_sb[:, mt, nt * P:(nt + 1) * P],
                            v01_sb[:, mt, h, :],
                            start=(mt == 0), stop=(mt == NT - 1),
                        )
                rdraw = spool.tile([P, NT], F32, tag="rdraw")
                nc.vector.reciprocal(rdraw[:, 0:4], halves[0][:, :, U])
                nc.vector.reciprocal(rdraw[:, 4:8], halves[1][:, :, U])
                rd09[h] = spool.tile([P, NT], F32, tag="rd09", name=f"rd09_{h}")
                rd9[h] = spool.tile([P, NT], F32, tag="rd9", name=f"rd9_{h}")
                nc.vector.tensor_scalar_mul(rd09[h][:], rdraw[:], 1.0 - ALPHA)
                nc.vector.tensor_scalar_mul(rd9[h][:], rdraw[:],
                                            (1.0 - ALPHA) / ALPHA)
                t = tpool.tile([P, NT, U], F32, tag="t")
                for i in (0, 1):
                    nc.vector.tensor_tensor(
                        t[:, 4 * i:4 * (i + 1), :],
                        halves[i][:, :, 0:U],
                        rd9[h][:, 4 * i:4 * (i + 1), None].to_broadcast([P, 4, U]),
                        mybir.AluOpType.mult,
                    )
                znew = zpool.tile([P, NT, U], BF16, tag="z")
                nc.vector.tensor_tensor(znew[:], t[:], v01_sb[:, :, h, 0:U],
                                        mybir.AluOpType.add)
                z_prev[h] = znew

            def hopk(h, hop):
                e_sb = e_tiles[h]
                hps = ps512.tile([P, NT, U], F32, tag="ps512")
                for nt in range(NT):
                    for mt in range(NT):
                        nc.tensor.matmul(
                            hps[:, nt, :],
                            e_sb[:, mt, nt * P:(nt + 1) * P],
                            z_prev[h][:, mt, :],
                            start=(mt == 0), stop=(mt == NT - 1),
                        )
                t = tpool.tile([P, NT, U], F32, tag="t")
                nc.vector.tensor_tensor(
                    t[:], hps[:],
                    rd09[h][:, :, None].to_broadcast([P, NT, U]),
                    mybir.AluOpType.mult,
                )
                if hop == 3:
                    out_ap = y_sb[:, :, h * U:(h + 1) * U]
                else:
                    znew = zpool.tile([P, NT, U], BF16, tag="z")
                    out_ap = znew[:]
                nc.vector.tensor_tensor(out_ap, t[:], v01_sb[:, :, h, 0:U],
                                        mybir.AluOpType.add)
                if hop != 3:
                    z_prev[h] = znew

            # ---- phase 1: projections, with pair-0 scores sprinkled in ----
            with tc.tile_pool(name="ph1", bufs=1) as p1:
                xT_sb = p1.tile([P, KT, N], BF16, tag="xT")
                nc.sync.dma_start(xT_sb[:], xT_d[:, :].rearrange("(t p) n -> p t n", p=P))
                wq_sb = p1.tile([P, KT, H], BF16, tag="wq")
                nc.gpsimd.dma_start(wq_sb[:], wq_d[:, :].rearrange("(t p) i -> p t i", p=P))
                wk_sb = p1.tile([P, KT, H], BF16, tag="wk")
                nc.scalar.dma_start(wk_sb[:], wk_d[:, :].rearrange("(t p) i -> p t i", p=P))
                wv_sb = p1.tile([P, KT, H], BF16, tag="wv")
                nc.scalar.dma_start(wv_sb[:], wv_d[:, :].rearrange("(t p) i -> p t i", p=P))

                def projqk_it(it):
                    for w_sb, b_sb, dst in ((wq_sb, bq_sb, qT_sb),
                                            (wk_sb, bk_sb, kT_sb)):
                        for ncx in range(2):
                            ps = ps512.tile([P, 512], F32, tag="ps512")
                            for kt in range(KT):
                                nc.tensor.matmul(
                                    ps[:],
                                    w_sb[:, kt, it * P:(it + 1) * P],
                                    xT_sb[:, kt, ncx * 512:(ncx + 1) * 512],
                                    start=(kt == 0), stop=(kt == KT - 1),
                                )
                            nc.vector.tensor_scalar_add(
                                dst[:, it, ncx * 512:(ncx + 1) * 512], ps[:],
                                b_sb[:, it:it + 1],
                            )

                def projv_nt(nt):
                    ps = ps512.tile([P, 512], F32, tag="ps512")
                    for kt in range(KT):
                        nc.tensor.matmul(
                            ps[:],
                            xT_sb[:, kt, nt * P:(nt + 1) * P],
                            wv_sb[:, kt, :],
                            start=(kt == 0), stop=(kt == KT - 1),
                        )
                    nc.vector.tensor_copy(
                        v01_sb[:, nt, :, 0:U],
                        ps[:].rearrange("p (h u) -> p h u", u=U),
                    )

                fillers = [(0, mt) for mt in range(NT)]
                projqk_it(0)
                for it in (1, 2, 3):
                    projqk_it(it)
                    scores_slot(*fillers.pop(0))
                for nt in range(NT):
                    projv_nt(nt)
                    if fillers:
                        scores_slot(*fillers.pop(0))

            # ---- phase 2: paired hops, next pair's scores sprinkled ----
            for p in range(NPAIR):
                a = 2 * p
                segs = [lambda h=a: hop0(h), lambda h=a + 1: hop0(h)]
                for hop in (1, 2, 3):
                    segs += [lambda h=a, k=hop: hopk(h, k),
                             lambda h=a + 1, k=hop: hopk(h, k)]
                for i, seg in enumerate(segs):
                    seg()
                    if p + 1 < NPAIR:
                        scores_slot(p + 1, i)

            # ---- phase 3: residual + LayerNorm ----
            for nt in range(NT):
                yt = y_sb[:, nt, :]
                nc.gpsimd.tensor_tensor(yt, yt, xn_sb[:, nt, :],
                                        mybir.AluOpType.add)
                st6 = spool.tile([P, 6], F32, tag="st6")
                nc.vector.bn_stats(st6[:], yt)
                st2 = spool.tile([P, 2], F32, tag="st2")
                nc.vector.bn_aggr(st2[:], st6[:])
                sd = spool.tile([P, 1], F32, tag="sd")
                nc.scalar.activation(
                    sd[:], st2[:, 1:2], mybir.ActivationFunctionType.Sqrt,
                    bias=eps_sb[:, :],
                )
                rstd = spool.tile([P, 1], F32, tag="rstd")
                nc.vector.reciprocal(rstd[:], sd[:])
                nc.vector.tensor_scalar(
                    yt, yt, st2[:, 0:1], rstd[:],
                    mybir.AluOpType.subtract, mybir.AluOpType.mult,
                )
                if apply_affine:
                    nc.vector.tensor_tensor(yt, yt, gam_sb[:, :], mybir.AluOpType.mult)
                    nc.vector.tensor_tensor(yt, yt, bet_sb[:, :], mybir.AluOpType.add)
                eng = nc.sync if nt % 2 == 0 else nc.scalar
                eng.dma_start(
                    out_d[:, :].rearrange("(t p) h -> p t h", p=P)[:, nt, :], yt)

    nc.finalize()
    _BUILD_CACHE[apply_affine] = nc
    return nc


def make_in_maps(x, adj, Wq, bq, Wk, bk, Wv, bv, gamma, beta, apply_affine):
    import ml_dtypes
    bf = ml_dtypes.bfloat16
    x = np.ascontiguousarray(np.asarray(x, np.float32))
    adj = np.asarray(adj)
    wqT = np.ascontiguousarray(np.asarray(Wq, np.float32).T.astype(bf))
    wkT = np.ascontiguousarray(np.asarray(Wk, np.float32).T.astype(bf))
    wvT01 = np.ascontiguousarray((ALPHA * np.asarray(Wv, np.float32)).T.astype(bf))
    bq = np.asarray(bq, np.float32)
    bk = np.asarray(bk, np.float32)
    bv = np.asarray(bv, np.float32)
    in_maps = []
    for b in range(B):
        m = {
            "xT": np.ascontiguousarray(x[b].T.astype(bf)),
            # +bv: P rows sum to 1, so the v-bias commutes through the hops
            # and lands as a constant row added to the residual input.
            "xn": x[b] + bv[None, :],
            "maskT": np.ascontiguousarray((adj[b] != 0).T.astype(bf)),
            "wqT": wqT, "wkT": wkT, "wvT01": wvT01,
            "bq": bq, "bk": bk,
        }
        if apply_affine:
            m["gammar"] = np.ascontiguousarray(
                np.broadcast_to(np.asarray(gamma, np.float32), (P, H)))
            m["betar"] = np.ascontiguousarray(
                np.broadcast_to(np.asarray(beta, np.float32), (P, H)))
        in_maps.append(m)
    return in_maps


def kernel(x, adj, Wq, bq, Wk, bk, Wv, bv, gamma, beta, _trace=False):
    apply_affine = not (
        np.allclose(np.asarray(gamma), 1.0) and np.allclose(np.asarray(beta), 0.0)
    )
    nc = build_nc(apply_affine)
    in_maps = make_in_maps(
        x, adj, Wq, bq, Wk, bk, Wv, bv, gamma, beta, apply_affine
    )
    res = run_bass_kernel_spmd(nc, in_maps, list(range(B)), trace=_trace)
    out = np.stack([np.asarray(res.results[b]["out"]) for b in range(B)])
    if _trace:
        return out.astype(np.float32), res
    return out.astype(np.float32)
